# revision 7
# baseline (speedup 1.0000x reference)
"""NoisyTopKRouter Trainium2 kernel.

Full inputs in, full outputs out; shards tokens across 8 NeuronCores.

Per-core dataflow (N_SH=2048 tokens, D=1024, E=64):
  host: xT = x_shard.T, epsT = eps_shard.T, Wcat = [route_w; noise_w].T
        (optional bf16 hi/lo split of xT and Wcat for 3-pass matmuls)
  device, per 512-token group g (processed in pairs to batch ACT table sets):
    psum[2E, 512] = sum_c WcatT_c.T @ xT_c
    ns    = ln(1 + exp(noise_logits + noise_b))    (ACT exp/ln, bias-folded)
    noisyT= (route_logits + route_b) + epsT * ns   (DVE, [E, 512])
    psumT[512 tok, E] = PE transpose of noisyT
    top2 via DVE max/max_index; probs = exp(noisy)*(noisy>=s2)/(e^s1+e^s2)
"""
import numpy as np

N, D, E = 16384, 1024, 64
NCORES = 8
N_SH = N // NCORES        # 2048 tokens per core
GSZ = 512                 # tokens per group
NG = N_SH // GSZ          # 4 groups
NSUB = GSZ // 128         # 4 subtiles per group
NCH = D // 128            # 8 contraction chunks
EC = 2 * E                # 128 = route|noise concatenated

MM_MODE = "bf16x3"        # "fp32" | "bf16x3"

_compiled = None


def _build():
    import concourse.bacc as bacc
    import concourse.mybir as mybir
    from concourse.tile import TileContext
    from concourse.masks import make_identity

    F32 = mybir.dt.float32
    BF16 = mybir.dt.bfloat16
    U32 = mybir.dt.uint32
    AF = mybir.ActivationFunctionType
    ALU = mybir.AluOpType

    nc = bacc.Bacc(None, target_bir_lowering=False, debug=False,
                   num_devices=NCORES)
    if MM_MODE == "fp32":
        x_ins = [nc.dram_tensor("xt", [D, N_SH], F32,
                                kind="ExternalInput").ap()]
        w_ins = [nc.dram_tensor("wc", [D, EC], F32,
                                kind="ExternalInput").ap()]
    else:
        x_ins = [nc.dram_tensor(n, [D, N_SH], BF16,
                                kind="ExternalInput").ap()
                 for n in ("xh", "xl")]
        w_ins = [nc.dram_tensor(n, [D, EC], BF16,
                                kind="ExternalInput").ap()
                 for n in ("wh", "wl")]
    bc_in = nc.dram_tensor("bc", [EC, 1], F32, kind="ExternalInput").ap()
    epst_in = nc.dram_tensor("epst", [E, N_SH], F32, kind="ExternalInput").ap()
    probs_out = nc.dram_tensor("probs", [N_SH, E], F32,
                               kind="ExternalOutput").ap()
    idx_out = nc.dram_tensor("idx", [N_SH, 2], U32, kind="ExternalOutput").ap()

    xdt = F32 if MM_MODE == "fp32" else BF16

    with TileContext(nc) as tc:
        with (
            tc.tile_pool(name="const", bufs=1) as cpool,
            tc.tile_pool(name="work", bufs=2) as pool,
            tc.tile_pool(name="xgp", bufs=3) as xpool,
            tc.tile_pool(name="psmm", bufs=3, space="PSUM") as psmm,
            tc.tile_pool(name="pstr", bufs=3, space="PSUM") as pstr,
            tc.tile_pool(name="pswarm", bufs=1, space="PSUM") as pswarm,
        ):
            ident = cpool.tile([128, 128], F32)
            make_identity(nc, ident[:])

            # HAM warmup: dummy matmuls during the initial DMA dead time
            pwarm = pswarm.tile([128, 128], F32, tag="warm")
            for _ in range(8):
                nc.tensor.matmul(pwarm[:], ident[:], ident[:],
                                 start=True, stop=True)

            # weights first on the sync (HWDGE/SP) ring
            wcs = []
            for wi, w_in in enumerate(w_ins):
                w = cpool.tile([128, NCH, EC], xdt, tag=f"wc{wi}")
                nc.sync.dma_start(out=w[:], in_=w_in.rearrange(
                    "(c p) m -> p c m", p=128))
                wcs.append(w)
            bc = cpool.tile([EC, 1], F32)
            nc.gpsimd.dma_start(out=bc[:], in_=bc_in)
            epst = cpool.tile([E, NG, GSZ], F32)
            nc.gpsimd.dma_start(out=epst[:], in_=epst_in.rearrange(
                "e (g n) -> e g n", g=NG))

            def load_xg(g):
                xgs = []
                for xi, x_in in enumerate(x_ins):
                    xg = xpool.tile([128, NCH, GSZ], xdt, tag=f"xg{xi}")
                    view = x_in[:, g * GSZ:(g + 1) * GSZ].rearrange(
                        "(c p) n -> p c n", p=128)
                    eng = nc.sync if (g + xi) % 2 == 0 else nc.gpsimd
                    if g == 0:
                        for c in range(NCH):
                            eng.dma_start(out=xg[:, c, :], in_=view[:, c, :])
                    else:
                        eng.dma_start(out=xg[:], in_=view)
                    xgs.append(xg)
                return xgs

            def matmuls(xgs):
                mm = psmm.tile([EC, GSZ], F32, tag="mm")
                if MM_MODE == "fp32":
                    for c in range(NCH):
                        nc.tensor.matmul(mm[:], wcs[0][:, c, :],
                                         xgs[0][:, c, :],
                                         start=(c == 0), stop=(c == NCH - 1))
                else:
                    wh, wl = wcs
                    xh, xl = xgs
                    for c in range(NCH):
                        # order: (wh,xh), (wh,xl), (wl,xh) — consecutive
                        # same-lhsT pairs minimize weight-load churn
                        nc.tensor.matmul(mm[:], wh[:, c, :], xh[:, c, :],
                                         start=(c == 0), stop=False)
                        nc.tensor.matmul(mm[:], wh[:, c, :], xl[:, c, :],
                                         start=False, stop=False)
                        nc.tensor.matmul(mm[:], wl[:, c, :], xh[:, c, :],
                                         start=False, stop=(c == NCH - 1))
                return mm

            def noise_exp(mm, g):
                ex1 = pool.tile([E, GSZ], F32, tag="ex1")
                nc.scalar.activation(ex1[:], mm[E:EC, :], AF.Exp,
                                     bias=bc[E:EC, 0:1])
                return ex1

            def noise_ln(ex1):
                ns = pool.tile([E, GSZ], F32, tag="ns")
                nc.scalar.activation(ns[:], ex1[:], AF.Ln, bias=1.0)
                return ns

            def group_epilogue(mm, ns, g, last):
                nm = pool.tile([E, GSZ], F32, tag="nm")
                nc.vector.tensor_mul(nm[:], epst[:, g, :], ns[:])
                rt = pool.tile([E, GSZ], F32, tag="rt")
                nc.vector.tensor_scalar(rt[:], mm[0:E, :], bc[0:E, 0:1], None,
                                        op0=ALU.add)
                noisyT = pool.tile([E, GSZ], F32, tag="noisyT")
                nc.vector.tensor_add(noisyT[:], rt[:], nm[:])

                tr = pstr.tile([128, NSUB, E], F32, tag="tr")
                for t in range(NSUB):
                    nc.tensor.transpose(tr[:, t],
                                        noisyT[:, t * 128:(t + 1) * 128],
                                        ident[0:E, 0:E])

                mx8 = pool.tile([128, NSUB, 8], F32, tag="mx8")
                ix8 = pool.tile([128, NSUB, 8], U32, tag="ix8")
                for t in range(NSUB):
                    nc.vector.max(out=mx8[:, t], in_=tr[:, t])
                    nc.vector.max_index(ix8[:, t], mx8[:, t], tr[:, t])

                e8 = pool.tile([128, NSUB, 8], F32, tag="e8")
                nc.scalar.activation(e8[:], mx8[:], AF.Exp)
                z4 = pool.tile([128, NSUB], F32, tag="z4")
                nc.vector.tensor_add(z4[:], e8[:, :, 0], e8[:, :, 1])
                rz4 = pool.tile([128, NSUB], F32, tag="rz4")
                nc.vector.reciprocal(rz4[:], z4[:])

                exv = pool.tile([128, NSUB, E], F32, tag="exv")
                nc.scalar.activation(exv[:], tr[:], AF.Exp)
                mrz = pool.tile([128, NSUB, E], F32, tag="mrz")
                for t in range(NSUB):
                    nc.vector.tensor_scalar(mrz[:, t], tr[:, t],
                                            mx8[:, t, 1:2], rz4[:, t:t + 1],
                                            op0=ALU.is_ge, op1=ALU.mult)
                prb = pool.tile([128, NSUB, E], F32, tag="prb")
                nc.vector.tensor_mul(prb[:], exv[:], mrz[:])

                eng = nc.sync if last else nc.scalar
                eng.dma_start(
                    out=probs_out[g * GSZ:(g + 1) * GSZ, :].rearrange(
                        "(t p) e -> p t e", p=128),
                    in_=prb[:])
                eng.dma_start(
                    out=idx_out[g * GSZ:(g + 1) * GSZ, :].rearrange(
                        "(t p) k -> p t k", p=128),
                    in_=ix8[:, :, 0:2])

            # process groups in pairs; batch ACT ops (E E L L ...) per pair
            # so walrus's greedy table-set picker stops thrashing exp<->ln
            for p in range(NG // 2):
                g0, g1 = 2 * p, 2 * p + 1
                xg0 = load_xg(g0)
                xg1 = load_xg(g1)
                mm0 = matmuls(xg0)
                mm1 = matmuls(xg1)
                ex0 = noise_exp(mm0, g0)
                ex1 = noise_exp(mm1, g1)
                ns0 = noise_ln(ex0)
                ns1 = noise_ln(ex1)
                group_epilogue(mm0, ns0, g0, last=False)
                group_epilogue(mm1, ns1, g1, last=(g1 == NG - 1))

    nc.compile()
    return nc


def _get_compiled():
    global _compiled
    if _compiled is None:
        _compiled = _build()
    return _compiled


def make_in_maps(x, route_w, route_b, noise_w, noise_b, eps):
    import ml_dtypes

    x = np.ascontiguousarray(np.asarray(x, dtype=np.float32))
    eps = np.ascontiguousarray(np.asarray(eps, dtype=np.float32))
    wc = np.ascontiguousarray(
        np.concatenate([np.asarray(route_w, dtype=np.float32),
                        np.asarray(noise_w, dtype=np.float32)], axis=0).T)
    bc = np.ascontiguousarray(
        np.concatenate([np.asarray(route_b, dtype=np.float32),
                        np.asarray(noise_b, dtype=np.float32)]).reshape(EC, 1))

    if MM_MODE != "fp32":
        wh = wc.astype(ml_dtypes.bfloat16)
        wl = (wc - wh.astype(np.float32)).astype(ml_dtypes.bfloat16)

    in_maps = []
    for c in range(NCORES):
        sl = slice(c * N_SH, (c + 1) * N_SH)
        xt = np.ascontiguousarray(x[sl].T)
        m = {"bc": bc, "epst": np.ascontiguousarray(eps[sl].T)}
        if MM_MODE == "fp32":
            m["xt"] = xt
            m["wc"] = wc
        else:
            xh = xt.astype(ml_dtypes.bfloat16)
            xlf = xt - xh.astype(np.float32)
            m["xh"] = np.ascontiguousarray(xh)
            m["xl"] = np.ascontiguousarray(xlf.astype(ml_dtypes.bfloat16))
            m["wh"] = wh
            m["wl"] = wl
        in_maps.append(m)
    return in_maps


def kernel(x, route_w, route_b, noise_w, noise_b, eps):
    from concourse.bass_utils import run_bass_kernel_spmd

    in_maps = make_in_maps(x, route_w, route_b, noise_w, noise_b, eps)
    nc = _get_compiled()
    res = run_bass_kernel_spmd(nc, in_maps, list(range(NCORES)))

    probs = np.concatenate([res.results[c]["probs"] for c in range(NCORES)], 0)
    idx = np.concatenate([res.results[c]["idx"] for c in range(NCORES)], 0)
    return probs, idx.view(np.int32)
